# revision 8
# baseline (speedup 1.0000x reference)
"""Trainium2 Bass kernel for nn_ContinuousEmbedding (masked matmul + bias).

Computes out = x @ (weights * mask) + bias, reshaped to [B, in_size, out_size],
where mask zeroes each input feature's own [out_size]-wide diagonal block.

Strategy: tensor-parallel across the 8 NeuronCores by splitting the
in_size*out_size (=16384) output columns into 8 shards of 2048 columns.
The mask is constant and folded into the weights on the host.

Per core the output is computed TRANSPOSED: out_t[oc, b] with oc (the 2048
output columns) on the partition axis and the 4096 batch rows on the free
axis; the host transposes back after gathering. This layout makes the bias
a per-partition scalar, so the PSUM eviction (+bias, fp32->fp16 cast) can
be split between the vector engine (tensor_scalar, ~658ns/tile) and the
scalar engine (activation Identity with bias, ~570ns/tile) instead of
bottlenecking one engine. It also makes the matmul's stationary operand a
weight slice reused across 4 consecutive matmuls (k-major accumulation
passes over 4 PSUM banks), amortizing LDWEIGHTS.

fp16 matmuls + fp16 output stores keep HBM traffic at ~19MB/core (~53us at
the 358GB/s/core limit), balanced against the ~55us matmul stream floor
(256 N=512 matmuls at 1 col/cycle, 2.4GHz warm). Stores alternate between
the sync HWDGE ring and the (otherwise idle) gpsimd SWDGE ring. The
accuracy budget (rel_err < 2e-2) has ~50x margin over fp16's ~4e-4.
"""

import numpy as np

B = 4096
IN_SIZE = 256
OUT_SIZE = 64
IO = IN_SIZE * OUT_SIZE          # 16384
N_CORES = 8
N_SHARD = IO // N_CORES          # 2048 output columns per core
P = 128                          # SBUF partitions
KO = IN_SIZE // P                # 2 contraction sub-tiles
B_TILE = 512                     # matmul moving free dim (one PSUM bank fp32)
J_TILES = N_SHARD // P           # 16 output-column tiles (partition dim)
B_TILES = B // B_TILE            # 8 batch chunks
HALF = B_TILES // 2              # 4 batch chunks per half

_CACHE: dict = {}


def _build_program():
    import concourse.mybir as mybir
    import concourse.tile as tile
    from concourse import bacc

    nc = bacc.Bacc(
        "TRN2", target_bir_lowering=False, debug=False, num_devices=N_CORES
    )
    dt16 = mybir.dt.float16
    f32 = mybir.dt.float32
    AF = mybir.ActivationFunctionType
    xt = nc.dram_tensor("xt", [IN_SIZE, B], dt16, kind="ExternalInput").ap()
    w = nc.dram_tensor("w", [IN_SIZE, N_SHARD], dt16, kind="ExternalInput").ap()
    # bias pre-transposed on host to [P, J_TILES] so column j is the
    # per-partition bias for output-column tile j.
    bias = nc.dram_tensor("bias", [P, J_TILES], f32, kind="ExternalInput").ap()
    # transposed output: [output columns, batch]
    out = nc.dram_tensor("out", [N_SHARD, B], dt16, kind="ExternalOutput").ap()

    with tile.TileContext(nc) as tc:
        with tc.tile_pool(name="const", bufs=1) as const, \
             tc.tile_pool(name="psum", bufs=2, space="PSUM") as psum_pool, \
             tc.tile_pool(name="outp", bufs=6) as outp:
            # Separate SBUF tiles per DMA chunk: Tile's dependency tracking
            # is tile-granular, so one big tile would make the first matmul
            # wait for ALL loads. Per-(k, j-half) w tiles and per-b x tiles
            # let compute start as soon as its own chunk lands.
            w_t = [[const.tile([P, N_SHARD // 2], dt16, name=f"w{k}{hj}",
                               tag=f"w{k}{hj}") for hj in range(2)]
                   for k in range(KO)]
            x_t = [const.tile([P, KO, B_TILE], dt16, name=f"x{b}", tag=f"x{b}")
                   for b in range(B_TILES)]
            bias_sb = const.tile([P, J_TILES], f32)

            w_src = w.rearrange("(ko p) n -> p ko n", p=P)
            xt_src = xt.rearrange("(ko p) m -> p ko m", p=P)
            # w + bias on the scalar HWDGE ring; x on the sync ring. The
            # rings drain concurrently, so the first matmul's inputs
            # (w[k0, first half] + x[b0]) arrive after ~0.5MB.
            H2 = N_SHARD // 2
            for hj in range(2):
                for k in range(KO):
                    nc.scalar.dma_start(
                        out=w_t[k][hj][:],
                        in_=w_src[:, k, hj * H2:(hj + 1) * H2])
            nc.scalar.dma_start(out=bias_sb[:], in_=bias[:])
            for b in range(B_TILES):
                nc.sync.dma_start(
                    out=x_t[b][:],
                    in_=xt_src[:, :, b * B_TILE:(b + 1) * B_TILE])

            # Batch-chunk phases: small first phase so dense matmul
            # streaming (and the HAM clock warm-up) starts as soon as
            # x[b0],x[b1] land; the big middle phase amortizes LDWEIGHTS
            # 4x; small last phase shortens the eviction+store tail.
            PHASES = [(0, 2), (2, 4), (6, 2)]
            st = 0  # store / eviction-engine alternator
            for b0, nb in PHASES:
                hb = slice(b0 * B_TILE, (b0 + nb) * B_TILE)
                for j in range(J_TILES):
                    js = slice(j * P, (j + 1) * P)
                    wj = slice((j % 8) * P, (j % 8 + 1) * P)
                    # One multi-bank PSUM tile per group: its nb 512-col
                    # bank slices accumulate independently, then ONE
                    # eviction op covers the whole group (amortizes the
                    # ~120-170cyc fixed cost per DVE/ACT op 4x).
                    ps = psum_pool.tile([P, 4 * B_TILE], f32,
                                        name="ps", tag="ps")
                    for k in range(KO):
                        for bi in range(nb):
                            nc.tensor.matmul(
                                ps[:, bi * B_TILE:(bi + 1) * B_TILE],
                                lhsT=w_t[k][j // 8][:, wj],
                                rhs=x_t[b0 + bi][:, k, :],
                                start=(k == 0),
                                stop=(k == KO - 1),
                            )
                    out_sb = outp.tile([P, nb * B_TILE], dt16,
                                       name=f"o{nb}", tag=f"o{nb}")
                    bj = bias_sb[:, j:j + 1]
                    nbs = slice(0, nb * B_TILE)
                    if st % 2 == 0:
                        nc.scalar.activation(
                            out_sb[:], ps[:, nbs], AF.Identity, bias=bj
                        )
                    else:
                        nc.vector.tensor_scalar(
                            out_sb[:], ps[:, nbs], bj, None,
                            op0=mybir.AluOpType.add,
                        )
                    # Stores: gpsimd (SWDGE) early while sync still loads x;
                    # then alternate; last phase on the two HWDGE rings
                    # (faster completion shortens the kernel tail).
                    if st < 16:
                        eng = nc.gpsimd
                    elif st < 32:
                        eng = nc.sync if st % 2 == 0 else nc.gpsimd
                    else:
                        eng = nc.sync if st % 2 == 0 else nc.scalar
                    st += 1
                    eng.dma_start(out=out[js, hb], in_=out_sb[:])

    nc.compile()
    return nc


def _get_program():
    if "prog" not in _CACHE:
        _CACHE["prog"] = _build_program()
    return _CACHE["prog"]


def _shard_inputs(x, weights, bias):
    # Fold the constant block-diagonal mask into the weights on the host.
    col_block = np.arange(IO, dtype=np.int64) // OUT_SIZE
    mask = (col_block[None, :] != np.arange(IN_SIZE)[:, None])
    wm = (weights * mask.astype(weights.dtype)).astype(np.float16)
    xt = np.ascontiguousarray(x.T.astype(np.float16))
    in_maps = []
    for c in range(N_CORES):
        sl = slice(c * N_SHARD, (c + 1) * N_SHARD)
        bias_t = np.ascontiguousarray(
            bias[sl].astype(np.float32).reshape(J_TILES, P).T
        )
        in_maps.append({
            "xt": xt,
            "w": np.ascontiguousarray(wm[:, sl]),
            "bias": bias_t,
        })
    return in_maps


def run_sharded(in_maps, **kwargs):
    """Run the SPMD program on cores 0-7. kwargs forwarded (e.g. trace)."""
    from concourse.bass_utils import run_bass_kernel_spmd

    nc = _get_program()
    return run_bass_kernel_spmd(
        nc, in_maps, core_ids=list(range(N_CORES)), **kwargs
    )


def kernel(x: np.ndarray, weights: np.ndarray, bias: np.ndarray) -> np.ndarray:
    x = np.asarray(x, dtype=np.float32)
    weights = np.asarray(weights, dtype=np.float32)
    bias = np.asarray(bias, dtype=np.float32)
    in_maps = _shard_inputs(x, weights, bias)
    res = run_sharded(in_maps)
    # Per-core outputs are [N_SHARD, B] (transposed); stack + transpose back.
    full_t = np.concatenate(
        [res.results[c]["out"] for c in range(N_CORES)], axis=0
    )  # [IO, B] fp16
    return np.ascontiguousarray(full_t.T).astype(np.float32).reshape(
        B, IN_SIZE, OUT_SIZE
    )
